# revision 1
# baseline (speedup 1.0000x reference)
"""BiLSTM-CRF loss kernel for trn2, one core = 32 sequences (data parallel).

Algorithm (validated in proto.py / proto_bf16.py):
- embedding gather via dma_gather(transpose) -> x (128=E, ntok) bf16, t-major
- BiLSTM: all-sigmoid gates (tanh(x)=2*sigma(2x)-1 folded into g-gate weights),
  gates psum accumulate: xproj window matmuls + rank-1 bias + per-step Whh mm
- emissions chunked: psum = WoutT halves @ H, EM = exp(psum + bout - log T)
- numerator: A' = sum OHM*(emis + end x laststep)  [TTR per chunk],
  B+C via (49,48) pair histogram (one-hot matmuls) . trans_ext
- CRF denominator in exp space: alpha chain (t=0..L/2-1) and backward G chain
  (t=L-1..L/2-1) with EEND x laststep injection; denom = sum log(dot) +
  log(T) * masksum
Output per core: (1, 8) f32: [0]=numerator partial sum, [1]=denominator partial.
loss = (sum_den - sum_num) / B   (host combines the 8 cores)
"""
import numpy as np
import ml_dtypes

import concourse.bacc as bacc
import concourse.mybir as mybir
from concourse.tile import TileContext

BF16 = ml_dtypes.bfloat16
F32 = np.float32
AF = mybir.ActivationFunctionType
ALU = mybir.AluOpType
DT = mybir.dt

T = 48


# --------------------------------------------------------------------------
# host-side preparation
# --------------------------------------------------------------------------

def prep_params(inp):
    """Build replicated parameter arrays (numpy) from raw inputs."""
    p = {}
    p["emb"] = np.ascontiguousarray(inp["emb"]).astype(BF16)

    def mk(Wih, Whh, bih, bhh):
        def reorder(W):
            i, f, g, o = np.split(np.asarray(W, F32), 4, 0)
            return np.concatenate([i, f, o, 2.0 * g], 0)
        WihT = np.ascontiguousarray(reorder(Wih).T).astype(BF16)   # (128, 512)
        WhhT = np.ascontiguousarray(reorder(Whh).T).astype(BF16)   # (128, 512)
        b = np.asarray(bih, F32) + np.asarray(bhh, F32)
        bi, bf_, bg, bo = np.split(b, 4)
        bias = np.concatenate([bi, bf_, bo, 2.0 * bg]).reshape(1, -1).astype(F32)
        return WihT, WhhT, bias

    p["wiht_f"], p["whht_f"], p["bias_f"] = mk(inp["Wih_f"], inp["Whh_f"], inp["bih_f"], inp["bhh_f"])
    p["wiht_b"], p["whht_b"], p["bias_b"] = mk(inp["Wih_b"], inp["Whh_b"], inp["bih_b"], inp["bhh_b"])
    Wout = np.asarray(inp["Wout"], F32)     # (48, 256)
    H = Wout.shape[1] // 2
    p["wot_f"] = np.ascontiguousarray(Wout[:, :H].T).astype(BF16)   # (128, 48)
    p["wot_b"] = np.ascontiguousarray(Wout[:, H:].T).astype(BF16)
    c0 = np.log(T)
    p["exbias"] = (np.asarray(inp["bout"], F32) - c0).reshape(T, 1).astype(F32)
    trans = np.asarray(inp["trans"], F32)
    p["et"] = np.exp(trans).astype(BF16)                     # (48,48) lhsT alpha
    p["ett"] = np.ascontiguousarray(np.exp(trans).T).astype(BF16)  # lhsT G
    p["estart"] = np.exp(np.asarray(inp["start_trans"], F32)).reshape(T, 1).astype(F32)
    p["eendrow"] = np.exp(np.asarray(inp["end_trans"], F32)).reshape(1, T).astype(BF16)
    p["endrow"] = np.asarray(inp["end_trans"], F32).reshape(1, T).astype(BF16)
    # bout is absent from the emission psum that the A-gather reads (it only
    # enters via the exp bias), so fold it in via the pair-histogram: every
    # masked position contributes exactly one CNT count with its cur-tag.
    p["transext"] = (np.concatenate(
        [trans, np.asarray(inp["start_trans"], F32)[None, :]], 0)
        + np.asarray(inp["bout"], F32)[None, :]).astype(F32)  # (49,48)
    p["iota48c"] = np.arange(T, dtype=F32).reshape(T, 1)
    p["iota64row"] = np.pad(np.arange(50, dtype=F32), (0, 14),
                            constant_values=-1.0).reshape(1, 64)
    p["iota48row"] = np.arange(T, dtype=F32).reshape(1, T)
    return p


def prep_shard(words, tags, mask):
    """Per-core input arrays. words/tags/mask: (b, L)."""
    b, L = words.shape
    ntok = b * L
    w_tm = np.ascontiguousarray(words.T).reshape(-1)
    tags_tm = np.ascontiguousarray(tags.T).reshape(-1)
    m_tm = np.ascontiguousarray(mask.T).reshape(-1).astype(F32)

    d = {}
    gi = w_tm.astype(np.int16).reshape(ntok // 16, 16).T          # (16, ntok/16)
    d["gidx"] = np.ascontiguousarray(np.tile(gi, (8, 1))).astype(np.int16)
    tm_masked = np.where(m_tm > 0, tags_tm, 99).astype(F32)
    d["tmask"] = tm_masked.astype(BF16).reshape(1, ntok)
    tprev = np.concatenate([np.full(b, 48, F32), tags_tm[:-b].astype(F32)])
    # pcol layouts (token%128 -> partition, token//128 -> free), host-prepped
    d["tags_pc"] = np.ascontiguousarray(tm_masked.reshape(-1, 128).T).astype(F32)
    d["tprev_pc"] = np.ascontiguousarray(tprev.reshape(-1, 128).T).astype(F32)
    d["mask_pc"] = np.ascontiguousarray(m_tm.reshape(-1, 128).T).astype(BF16)
    m_pad = np.pad(m_tm, (0, b))
    d["lsrow"] = (m_tm - m_pad[b:]).astype(BF16).reshape(1, ntok)
    return d


# --------------------------------------------------------------------------
# device kernel builder
# --------------------------------------------------------------------------

def build(L=512, BLOC=32, W=8, V=32000, debug=False, phases=("lstm", "hist", "emis", "crf")):
    ntok = L * BLOC
    NW = L // W
    half = L // 2
    NCH = ntok // 512          # emission chunks
    NPCH = ntok // 128         # one-hot pchunks
    c0 = float(np.log(T))

    nc = bacc.Bacc()
    dp = nc.declare_dram_parameter
    g_gidx = dp("gidx", [128, ntok // 16], DT.int16, isOutput=False)
    g_tmask = dp("tmask", [1, ntok], DT.bfloat16, isOutput=False)
    g_tagspc = dp("tags_pc", [128, ntok // 128], DT.float32, isOutput=False)
    g_tprevpc = dp("tprev_pc", [128, ntok // 128], DT.float32, isOutput=False)
    g_maskpc = dp("mask_pc", [128, ntok // 128], DT.bfloat16, isOutput=False)
    g_lsrow = dp("lsrow", [1, ntok], DT.bfloat16, isOutput=False)
    g_emb = dp("emb", [V, 128], DT.bfloat16, isOutput=False)
    g_w = {}
    for nm in ("wiht_f", "whht_f", "wiht_b", "whht_b"):
        g_w[nm] = dp(nm, [128, 512], DT.bfloat16, isOutput=False)
    g_bias = {d: dp(f"bias_{d}", [1, 512], DT.float32, isOutput=False) for d in "fb"}
    g_wot = {d: dp(f"wot_{d}", [128, T], DT.bfloat16, isOutput=False) for d in "fb"}
    g_exbias = dp("exbias", [T, 1], DT.float32, isOutput=False)
    g_et = dp("et", [T, T], DT.bfloat16, isOutput=False)
    g_ett = dp("ett", [T, T], DT.bfloat16, isOutput=False)
    g_estart = dp("estart", [T, 1], DT.float32, isOutput=False)
    g_eendrow = dp("eendrow", [1, T], DT.bfloat16, isOutput=False)
    g_endrow = dp("endrow", [1, T], DT.bfloat16, isOutput=False)
    g_transext = dp("transext", [49, T], DT.float32, isOutput=False)
    g_iota48c = dp("iota48c", [T, 1], DT.float32, isOutput=False)
    g_iota64row = dp("iota64row", [1, 64], DT.float32, isOutput=False)
    g_iota48row = dp("iota48row", [1, T], DT.float32, isOutput=False)
    g_out = dp("out", [1, 8], DT.float32, isOutput=True)
    if debug:
        g_dbg1 = dp("dbg1", [T, 512], DT.float32, isOutput=True)
        g_dbg2 = dp("dbg2", [T, 512], DT.float32, isOutput=True)
        g_dbg3 = dp("dbg3", [T, 16], DT.float32, isOutput=True)

    with TileContext(nc) as tc:
        with tc.tile_pool(name="persist", bufs=1) as pp:
            # ---- persistent SBUF tiles
            Hf = pp.tile([128, ntok], DT.bfloat16, tag="Hf", name="Hf")
            Hb = pp.tile([128, ntok], DT.bfloat16, tag="Hb", name="Hb")
            wiht = {}
            whht = {}
            bias = {}
            wot = {}
            for d in "fb":
                wiht[d] = pp.tile([128, 512], DT.bfloat16, tag=f"wiht{d}", name=f"wiht{d}")
                whht[d] = pp.tile([128, 512], DT.bfloat16, tag=f"whht{d}", name=f"whht{d}")
                bias[d] = pp.tile([1, 512], DT.float32, tag=f"bias{d}", name=f"bias{d}")
                wot[d] = pp.tile([128, T], DT.bfloat16, tag=f"wot{d}", name=f"wot{d}")
            exbias = pp.tile([T, 1], DT.float32, tag="exbias", name="exbias")
            et_sb = pp.tile([T, T], DT.bfloat16, tag="et", name="et")
            ett_sb = pp.tile([T, T], DT.bfloat16, tag="ett", name="ett")
            estart = pp.tile([T, 1], DT.float32, tag="estart", name="estart")
            eendrow = pp.tile([1, T], DT.bfloat16, tag="eendrow", name="eendrow")
            endrow = pp.tile([1, T], DT.bfloat16, tag="endrow", name="endrow")
            transext = pp.tile([49, T], DT.float32, tag="transext", name="transext")
            iota48c = pp.tile([T, 1], DT.float32, tag="iota48c", name="iota48c")
            iota64row = pp.tile([1, 64], DT.float32, tag="iota64row", name="iota64row")
            iota48row = pp.tile([1, T], DT.float32, tag="iota48row", name="iota48row")
            tmask_sb = pp.tile([1, ntok], DT.bfloat16, tag="tmask", name="tmask")
            tags_pcol = pp.tile([128, NPCH], DT.float32, tag="tagspcol", name="tagspcol")
            tprev_pcol = pp.tile([128, NPCH], DT.float32, tag="tprevpcol", name="tprevpcol")
            m_pcol = pp.tile([128, NPCH], DT.bfloat16, tag="mpcol", name="mpcol")
            lsrow = pp.tile([1, ntok], DT.bfloat16, tag="lsrow", name="lsrow")
            # small constants
            ones48row = pp.tile([1, T], DT.float32, tag="ones48row", name="ones48row")
            ones48rowb = pp.tile([1, T], DT.bfloat16, tag="ones48rowb", name="ones48rowb")
            ones128row = pp.tile([1, 128], DT.float32, tag="ones128row", name="ones128row")
            onesrow512 = pp.tile([1, 512], DT.float32, tag="onesrow512", name="onesrow512")
            ones48col = pp.tile([T, 1], DT.float32, tag="ones48col", name="ones48col")
            ones49col = pp.tile([49, 1], DT.float32, tag="ones49col", name="ones49col")
            ones128col = pp.tile([128, 1], DT.float32, tag="ones128col", name="ones128col")
            iota64b = pp.tile([128, 64], DT.float32, tag="iota64b", name="iota64b")
            iota48b = pp.tile([128, T], DT.float32, tag="iota48b", name="iota48b")
            # LSTM state
            cst = {d: pp.tile([128, BLOC], DT.float32, tag=f"c{d}", name=f"c{d}") for d in "fb"}
            tmp1 = {d: pp.tile([128, BLOC], DT.float32, tag=f"tmp1{d}", name=f"tmp1{d}") for d in "fb"}
            tmp2 = {d: pp.tile([128, BLOC], DT.bfloat16, tag=f"tmp2{d}", name=f"tmp2{d}") for d in "fb"}
            tct = {d: pp.tile([128, BLOC], DT.bfloat16, tag=f"tct{d}", name=f"tct{d}") for d in "fb"}
            jacc = {d: pp.tile([128, 1], DT.float32, tag=f"jacc{d}", name=f"jacc{d}") for d in "fb"}
            # numerator accumulators
            accA = pp.tile([T, NCH], DT.float32, tag="accA", name="accA")
            accA_red = pp.tile([T, 1], DT.float32, tag="accAred", name="accAred")
            accBC = pp.tile([49, 1], DT.float32, tag="accBC", name="accBC")
            junkA = pp.tile([T, 512], DT.bfloat16, tag="junkA", name="junkA")
            junkBC = pp.tile([49, T], DT.float32, tag="junkBC", name="junkBC")
            msum = pp.tile([128, 1], DT.float32, tag="msum", name="msum")
            # CRF tiles
            ea = [pp.tile([T, BLOC], DT.bfloat16, tag=f"ea{i}", name=f"ea{i}") for i in range(2)]
            emg = pp.tile([T, BLOC], DT.bfloat16, tag="emg", name="emg")
            dott = pp.tile([T, BLOC], DT.float32, tag="dott", name="dott")
            logrow = pp.tile([1, BLOC], DT.float32, tag="logrow", name="logrow")
            dsum = pp.tile([1, 1], DT.float32, tag="dsum", name="dsum")
            tmp11 = pp.tile([1, 1], DT.float32, tag="tmp11", name="tmp11")
            out_sb = pp.tile([1, 8], DT.float32, tag="outsb", name="outsb")

            # ---- input DMAs
            S = nc.sync
            for d in "fb":
                S.dma_start(out=wiht[d][:], in_=g_w[f"wiht_{d}"][:])
                S.dma_start(out=whht[d][:], in_=g_w[f"whht_{d}"][:])
                S.dma_start(out=bias[d][:], in_=g_bias[d][:])
                S.dma_start(out=wot[d][:], in_=g_wot[d][:])
            S.dma_start(out=exbias[:], in_=g_exbias[:])
            S.dma_start(out=et_sb[:], in_=g_et[:])
            S.dma_start(out=ett_sb[:], in_=g_ett[:])
            S.dma_start(out=estart[:], in_=g_estart[:])
            S.dma_start(out=eendrow[:], in_=g_eendrow[:])
            S.dma_start(out=endrow[:], in_=g_endrow[:])
            S.dma_start(out=transext[:], in_=g_transext[:])
            S.dma_start(out=iota48c[:], in_=g_iota48c[:])
            S.dma_start(out=iota64row[:], in_=g_iota64row[:])
            S.dma_start(out=iota48row[:], in_=g_iota48row[:])
            S.dma_start(out=tmask_sb[:], in_=g_tmask[:])
            S.dma_start(out=tags_pcol[:], in_=g_tagspc[:])
            S.dma_start(out=tprev_pcol[:], in_=g_tprevpc[:])
            S.dma_start(out=m_pcol[:], in_=g_maskpc[:])
            S.dma_start(out=lsrow[:], in_=g_lsrow[:])

            # constants
            Vv = nc.vector
            Sc = nc.scalar
            Vv.memset(ones48row[:], 1.0)
            Vv.memset(ones48rowb[:], 1.0)
            Vv.memset(ones128row[:], 1.0)
            Vv.memset(onesrow512[:], 1.0)
            Vv.memset(ones48col[:], 1.0)
            Vv.memset(ones49col[:], 1.0)
            Vv.memset(ones128col[:], 1.0)
            Vv.memset(accA[:], 0.0)
            Vv.memset(out_sb[:], 0.0)
            for d in "fb":
                Vv.memset(cst[d][:], 0.0)

            with tc.tile_pool(name="iotaps", bufs=1, space="PSUM") as ipsp:
                ip = ipsp.tile([128, 64], DT.float32, name="ip")
                nc.tensor.matmul(ip[:], ones128row[:], iota64row[:], start=True, stop=True)
                Vv.tensor_copy(iota64b[:], ip[:])
                ip2 = ipsp.tile([128, T], DT.float32, name="ip2")
                nc.tensor.matmul(ip2[:], ones128row[:], iota48row[:], start=True, stop=True)
                Vv.tensor_copy(iota48b[:], ip2[:])

            # ---------------- LSTM ----------------
            emis_lvl = 4
            for ph in phases:
                if ph.startswith("emis") and len(ph) > 4:
                    emis_lvl = int(ph[4:])
            do_lstm = "lstm" in phases
            do_hist = "hist" in phases
            do_emis = "emis" in phases
            do_crf = "crf" in phases
            do_emis = do_emis or any(ph.startswith("emis") for ph in phases)
            if not do_lstm:
                Vv.memset(Hf[:], 0.0)
                Vv.memset(Hb[:], 0.0)
            REG = 32 * W      # region width per gate
            Hdir = {"f": Hf, "b": Hb}
            with tc.tile_pool(name="lstm_ps", bufs=2, space="PSUM") as lpsp, \
                 tc.tile_pool(name="lstm_sb", bufs=3) as lsb, \
                 tc.tile_pool(name="xpool", bufs=1) as xp:
                x = xp.tile([128, ntok], DT.bfloat16, tag="x", name="x")
                gidx = xp.tile([128, ntok // 16], DT.int16, tag="gidx", name="gidx")
                S.dma_start(out=gidx[:], in_=g_gidx[:])
                GCH = min(ntok, 1024)
                _ng = ntok // GCH
                _order = []
                for _i in range((_ng + 1) // 2):
                    _order.append(_i)
                    if _ng - 1 - _i != _i:
                        _order.append(_ng - 1 - _i)
                for gc in _order:
                    nc.gpsimd.dma_gather(
                        out_ap=x[:, gc * GCH:(gc + 1) * GCH].rearrange(
                            "p (o n) -> p o n", o=1),
                        in_ap=g_emb[:],
                        idxs_ap=gidx[:, gc * (GCH // 16):(gc + 1) * (GCH // 16)],
                        num_idxs=GCH,
                        num_idxs_reg=GCH,
                        elem_size=128,
                        transpose=True,
                        single_packet=False,
                    )
                for w in range(NW if do_lstm else 0):
                    pf = {}
                    for d in "fb":
                        pf[d] = lpsp.tile([128, 4 * REG], DT.float32, tag=f"pf{d}", name=f"pf{d}")
                        if d == "f":
                            x0 = w * W * BLOC
                        else:
                            x0 = (L - (w + 1) * W) * BLOC
                        for gi in range(4):
                            nc.tensor.matmul(
                                pf[d][:, gi * REG:(gi + 1) * REG],
                                wiht[d][:, gi * 128:(gi + 1) * 128],
                                x[:, x0:x0 + W * BLOC],
                                start=((gi * REG * 4) % 2048 == 0),
                                stop=False, skip_group_check=True)
                            nc.tensor.matmul(
                                pf[d][:, gi * REG:(gi + 1) * REG],
                                bias[d][0:1, gi * 128:(gi + 1) * 128],
                                onesrow512[0:1, 0:W * BLOC],
                                start=False, stop=False, skip_group_check=True)
                    for s in range(W):
                        for d in "fb":
                            if d == "f":
                                t = w * W + s
                                slot = s
                                tprev_col = (t - 1) * BLOC
                                first = (t == 0)
                            else:
                                t = L - 1 - (w * W + s)
                                slot = W - 1 - s
                                tprev_col = (t + 1) * BLOC
                                first = (t == L - 1)
                            Hd = Hdir[d]
                            pfd = pf[d]
                            if not first:
                                for gi in range(4):
                                    nc.tensor.matmul(
                                        pfd[:, gi * REG + slot * 32: gi * REG + (slot + 1) * 32],
                                        whht[d][:, gi * 128:(gi + 1) * 128],
                                        Hd[:, tprev_col:tprev_col + BLOC],
                                        start=False, stop=True, skip_group_check=True)
                            # sigma over the 4 gate slices
                            Sg = lsb.tile([128, 128], DT.bfloat16, tag=f"S{d}", name=f"S{d}")
                            pf3 = pfd[:].rearrange("p (g n) -> p g n", g=4)
                            Sc.activation(
                                Sg[:].rearrange("p (g n) -> p g n", g=4),
                                pf3[:, :, slot * 32:(slot + 1) * 32],
                                AF.Sigmoid)
                            # c update
                            if first:
                                Vv.affine_mul_reduce(
                                    out=tmp2[d][:], accum_out=jacc[d][:],
                                    in0=Sg[:, 96:128], in1=Sg[:, 0:32],
                                    scale=2.0, bias=-1.0)
                                Vv.tensor_copy(cst[d][:], tmp2[d][:])
                            else:
                                Vv.tensor_tensor(out=tmp1[d][:], in0=Sg[:, 32:64],
                                                 in1=cst[d][:], op=ALU.mult)
                                Vv.affine_mul_reduce(
                                    out=tmp2[d][:], accum_out=jacc[d][:],
                                    in0=Sg[:, 96:128], in1=Sg[:, 0:32],
                                    scale=2.0, bias=-1.0)
                                Vv.tensor_tensor(out=cst[d][:], in0=tmp1[d][:],
                                                 in1=tmp2[d][:], op=ALU.add)
                            Sc.activation(tct[d][:], cst[d][:], AF.Tanh)
                            Vv.tensor_tensor(out=Hd[:, t * BLOC:(t + 1) * BLOC],
                                             in0=Sg[:, 64:96], in1=tct[d][:],
                                             op=ALU.mult)

            # ---------------- one-hot histogram (B + C numerator part) -------
            empool = tc.tile_pool(name="empool", bufs=1)
            emp = empool.__enter__()
            EM = emp.tile([T, ntok], DT.bfloat16, tag="EM", name="EM")
            if not do_emis:
                Vv.memset(EM[:], 0.01)
            with tc.tile_pool(name="cnt_ps", bufs=1, space="PSUM") as cpsp, \
                 tc.tile_pool(name="oht_sb", bufs=3) as osb:
                cntps = cpsp.tile([64, T], DT.float32, name="cntps")
                if not do_hist:
                    Vv.memset(accBC[:], 0.0)
                for q in range(NPCH if do_hist else 0):
                    ohp = osb.tile([128, 64], DT.bfloat16, tag="ohp", name="ohp")
                    ohc = osb.tile([128, T], DT.bfloat16, tag="ohc", name="ohc")
                    Vv.tensor_scalar(ohp[:], iota64b[:], tprev_pcol[:, q:q + 1],
                                     None, ALU.is_equal)
                    Vv.tensor_scalar(ohc[:], iota48b[:], tags_pcol[:, q:q + 1],
                                     None, ALU.is_equal)
                    nc.tensor.matmul(cntps[:], ohp[:], ohc[:],
                                     start=(q == 0), stop=(q == NPCH - 1),
                                     skip_group_check=True)
                if do_hist:
                    Vv.affine_mul_reduce(
                        out=junkBC[:], accum_out=accBC[:],
                        in0=transext[:], in1=cntps[0:49, :],
                        scale=1.0, bias=0.0)

            # ---------------- emissions + A-part numerator ----------------
            with tc.tile_pool(name="em_ps", bufs=2, space="PSUM") as epsp, \
                 tc.tile_pool(name="em_sb", bufs=3) as esb:
                for k in range(NCH if do_emis else 0):
                    cs = k * 512
                    emps = epsp.tile([T, 512], DT.float32, tag="emps", name="emps")
                    nc.tensor.matmul(emps[:], wot["f"][:], Hf[:, cs:cs + 512],
                                     start=True, stop=False, skip_group_check=True)
                    nc.tensor.matmul(emps[:], wot["b"][:], Hb[:, cs:cs + 512],
                                     start=False, stop=False, skip_group_check=True)
                    Sc.activation(EM[:, cs:cs + 512], emps[:], AF.Exp,
                                  bias=exbias[:])
                    if emis_lvl >= 2:
                        # fold end_trans * laststep into psum for the A-gather
                        nc.tensor.matmul(
                            emps[:], endrow[:],
                            lsrow[0:1, cs:cs + 512],
                            start=False, stop=True, skip_group_check=True)
                    if emis_lvl >= 3:
                        # tags broadcast + one-hot
                        tgps = epsp.tile([T, 512], DT.float32, tag="tgps", name="tgps")
                        nc.tensor.matmul(tgps[:], ones48rowb[:],
                                         tmask_sb[0:1, cs:cs + 512], start=True, stop=True,
                                         skip_group_check=True)
                        ohm = esb.tile([T, 512], DT.bfloat16, tag="ohm", name="ohm")
                        Vv.tensor_scalar(ohm[:], tgps[:], iota48c[:], None, ALU.is_equal)
                    if emis_lvl >= 4:
                        Vv.affine_mul_reduce(
                            out=junkA[:], accum_out=accA[:, k:k + 1],
                            in0=emps[:], in1=ohm[:],
                            scale=1.0, bias=0.0)
                    if debug and k == NCH - 1:
                        demp = esb.tile([T, 512], DT.float32, name="demp")
                        Vv.tensor_copy(demp[:], emps[:])
                        S.dma_start(out=g_dbg1[:], in_=demp[:])
                        dohm = esb.tile([T, 512], DT.float32, name="dohm")
                        Vv.tensor_copy(dohm[:], ohm[:])
                        S.dma_start(out=g_dbg2[:], in_=dohm[:])

            # ---------------- CRF ----------------
            with tc.tile_pool(name="crf_ps", bufs=2, space="PSUM") as kpsp, \
                 tc.tile_pool(name="fin_ps", bufs=1, space="PSUM") as fpsp:
                # alpha chain
                Vv.tensor_scalar(ea[0][:], EM[:, 0:BLOC], estart[:], None, ALU.mult)
                cur = 0
                for t in range(1, half if do_crf else 1):
                    pa = kpsp.tile([T, BLOC], DT.float32, tag="pa", name="pa")
                    nc.tensor.matmul(pa[:], et_sb[:], ea[cur][:], start=True, stop=True,
                                     skip_group_check=True)
                    cur ^= 1
                    Vv.tensor_tensor(out=ea[cur][:], in0=pa[:],
                                     in1=EM[:, t * BLOC:(t + 1) * BLOC], op=ALU.mult)
                # G chain: t goes L-1 down to half-1; G_t kept in psum
                def ls_slice(t):
                    tok = t * BLOC
                    return lsrow[0:1, tok:tok + BLOC]

                gps_prev = kpsp.tile([T, BLOC], DT.float32, tag="pg", name="pg")
                nc.tensor.matmul(gps_prev[:], eendrow[:], ls_slice(L - 1),
                                 start=True, stop=True, skip_group_check=True)
                for t in range(L - 2, (half - 2) if do_crf else (L - 2), -1):
                    Vv.tensor_tensor(out=emg[:], in0=gps_prev[:],
                                     in1=EM[:, (t + 1) * BLOC:(t + 2) * BLOC],
                                     op=ALU.mult)
                    gps = kpsp.tile([T, BLOC], DT.float32, tag="pg", name="pg")
                    nc.tensor.matmul(gps[:], ett_sb[:], emg[:], start=True, stop=False,
                                     skip_group_check=True)
                    nc.tensor.matmul(gps[:], eendrow[:], ls_slice(t),
                                     start=False, stop=True, skip_group_check=True)
                    gps_prev = gps
                # combine
                Vv.tensor_tensor(out=dott[:], in0=gps_prev[:], in1=ea[cur][:],
                                 op=ALU.mult)
                fint = fpsp.tile([1, 64], DT.float32, name="fint")
                nc.tensor.matmul(fint[:, 0:BLOC], ones48col[:], dott[:], start=True,
                                 stop=True, skip_group_check=True)
                Sc.activation(logrow[:], fint[:, 0:BLOC], AF.Ln)
                Vv.tensor_reduce(dsum[:], logrow[:], mybir.AxisListType.X, ALU.add)

                # masksum
                Vv.tensor_reduce(msum[:], m_pcol[:], mybir.AxisListType.X, ALU.add)
                nc.tensor.matmul(fint[:, 32:33], msum[:], ones128col[:], start=True,
                                 stop=True, skip_group_check=True)
                # numerator total: A (start) + BC (accumulate) in one cell
                Vv.tensor_reduce(accA_red[:], accA[:], mybir.AxisListType.X, ALU.add)
                nc.tensor.matmul(fint[:, 34:35], accA_red[:], ones48col[:], start=True,
                                 stop=False, skip_group_check=True)
                nc.tensor.matmul(fint[:, 34:35], accBC[:], ones49col[:], start=False,
                                 stop=True, skip_group_check=True)
                # out[0] = numsum ; out[1] = denomsum
                Vv.tensor_copy(out_sb[:, 0:1], fint[:, 34:35])
                Vv.tensor_scalar(tmp11[:], fint[:, 32:33], c0, None, ALU.mult)
                Vv.tensor_tensor(out=out_sb[:, 1:2], in0=tmp11[:], in1=dsum[:],
                                 op=ALU.add)
                Vv.tensor_copy(out_sb[:, 4:5], fint[:, 32:33])
                Vv.tensor_copy(out_sb[:, 5:6], dsum[:])
            if debug:
                daccA = pp.tile([T, 16], DT.float32, tag="daccA", name="daccA")
                Vv.memset(daccA[:], 0.0)
                Vv.tensor_copy(daccA[:, 0:NCH if NCH <= 16 else 16],
                               accA[:, 0:NCH if NCH <= 16 else 16])
                S.dma_start(out=g_dbg3[:], in_=daccA[:])
            empool.__exit__(None, None, None)
            S.dma_start(out=g_out[:], in_=out_sb[:])

    return nc


# --------------------------------------------------------------------------
# self-contained entry point: kernel(**inputs) -> scalar loss (numpy)
# --------------------------------------------------------------------------

_CACHED = {}


def _get_nc():
    if "nc" not in _CACHED:
        nc = build(L=512, BLOC=32, W=8, V=32000)
        if not nc.is_finalized():
            nc.finalize()
        _CACHED["nc"] = nc
    return _CACHED["nc"]


def kernel(**inputs):
    from concourse.bass_utils import run_bass_kernel_spmd

    B = 256
    BLOC = B // 8
    p = prep_params(inputs)
    in_maps = []
    words = np.asarray(inputs["words"])
    tags = np.asarray(inputs["tags"])
    mask = np.asarray(inputs["mask"])
    for core in range(8):
        sl = slice(core * BLOC, (core + 1) * BLOC)
        d = prep_shard(words[sl], tags[sl], mask[sl])
        d.update(p)
        in_maps.append(d)
    nc = _get_nc()
    res = run_bass_kernel_spmd(nc, in_maps, list(range(8)))
    tot_num = sum(float(res.results[i]["out"][0, 0]) for i in range(8))
    tot_den = sum(float(res.results[i]["out"][0, 1]) for i in range(8))
    loss = (tot_den - tot_num) / B
    return np.float32(loss)



# revision 19
# speedup vs baseline: 2.6165x; 2.6165x over previous
"""BiLSTM-CRF loss kernel for trn2, one core = 32 sequences (data parallel).

Algorithm (validated in proto.py / proto_bf16.py):
- embedding gather via dma_gather(transpose) -> x (128=E, ntok) bf16, t-major
- BiLSTM: all-sigmoid gates (tanh(x)=2*sigma(2x)-1 folded into g-gate weights),
  gates psum accumulate: xproj window matmuls + rank-1 bias + per-step Whh mm
- emissions chunked: psum = WoutT halves @ H, EM = exp(psum + bout - log T)
- numerator: A' = sum OHM*(emis + end x laststep)  [TTR per chunk],
  B+C via (49,48) pair histogram (one-hot matmuls) . trans_ext
- CRF denominator in exp space: alpha chain (t=0..L/2-1) and backward G chain
  (t=L-1..L/2-1) with EEND x laststep injection; denom = sum log(dot) +
  log(T) * masksum
Output per core: (1, 8) f32: [0]=numerator partial sum, [1]=denominator partial.
loss = (sum_den - sum_num) / B   (host combines the 8 cores)
"""
import numpy as np
import ml_dtypes

import concourse.bacc as bacc
import concourse.mybir as mybir
from concourse.tile import TileContext

BF16 = ml_dtypes.bfloat16
F32 = np.float32
AF = mybir.ActivationFunctionType
ALU = mybir.AluOpType
DT = mybir.dt

T = 48


# --------------------------------------------------------------------------
# host-side preparation
# --------------------------------------------------------------------------

def prep_params(inp):
    """Build replicated parameter arrays (numpy) from raw inputs."""
    p = {}
    p["emb"] = np.ascontiguousarray(inp["emb"]).astype(BF16)

    def mk(Wih, Whh, bih, bhh):
        def reorder(W):
            i, f, g, o = np.split(np.asarray(W, F32), 4, 0)
            return np.concatenate([i, f, o, 2.0 * g], 0)
        WihT = np.ascontiguousarray(reorder(Wih).T).astype(BF16)   # (128, 512)
        WhhT = np.ascontiguousarray(reorder(Whh).T).astype(BF16)   # (128, 512)
        b = np.asarray(bih, F32) + np.asarray(bhh, F32)
        bi, bf_, bg, bo = np.split(b, 4)
        bias = np.concatenate([bi, bf_, bo, 2.0 * bg]).reshape(1, -1).astype(F32)
        return WihT, WhhT, bias

    p["wiht_f"], p["whht_f"], p["bias_f"] = mk(inp["Wih_f"], inp["Whh_f"], inp["bih_f"], inp["bhh_f"])
    p["wiht_b"], p["whht_b"], p["bias_b"] = mk(inp["Wih_b"], inp["Whh_b"], inp["bih_b"], inp["bhh_b"])
    # segmented-LSTM bias operands: lhsT (4,128) per dir + block-onehot rhs
    SW = 8 * 32
    for dd in "fb":
        p[f"bias4_{dd}"] = np.ascontiguousarray(
            p.pop(f"bias_{dd}").reshape(4, 128)).astype(BF16)
    blk = np.zeros((4, 4 * SW), F32)
    for c in range(4):
        blk[c, c * SW:(c + 1) * SW] = 1.0
    p["blk1"] = blk.astype(BF16)
    Wout = np.asarray(inp["Wout"], F32)     # (48, 256)
    H = Wout.shape[1] // 2
    p["wot_f"] = np.ascontiguousarray(Wout[:, :H].T).astype(BF16)   # (128, 48)
    p["wot_b"] = np.ascontiguousarray(Wout[:, H:].T).astype(BF16)
    c0 = np.log(T)
    p["exbias"] = (np.asarray(inp["bout"], F32) - c0).reshape(T, 1).astype(F32)
    trans = np.asarray(inp["trans"], F32)
    p["et"] = np.exp(trans).astype(BF16)                     # (48,48) lhsT alpha
    p["ett"] = np.ascontiguousarray(np.exp(trans).T).astype(BF16)  # lhsT G
    p["estart"] = np.exp(np.asarray(inp["start_trans"], F32)).reshape(T, 1).astype(F32)
    p["eendrow"] = np.exp(np.asarray(inp["end_trans"], F32)).reshape(1, T).astype(BF16)
    p["endrow"] = np.asarray(inp["end_trans"], F32).reshape(1, T).astype(BF16)
    # bout is absent from the emission psum that the A-gather reads (it only
    # enters via the exp bias), so fold it in via the pair-histogram: every
    # masked position contributes exactly one CNT count with its cur-tag.
    p["transext"] = (np.concatenate(
        [trans, np.asarray(inp["start_trans"], F32)[None, :]], 0)
        + np.asarray(inp["bout"], F32)[None, :]).astype(F32)  # (49,48)
    p["iota48c"] = np.arange(T, dtype=F32).reshape(T, 1)
    p["iota64row"] = np.pad(np.arange(50, dtype=F32), (0, 14),
                            constant_values=-1.0).reshape(1, 64)
    p["iota48row"] = np.arange(T, dtype=F32).reshape(1, T)
    return p


NSEG = 8
BURN = 16


def prep_shard(words, tags, mask, emb_bf):
    """Per-core input arrays. words/tags/mask: (b, L)."""
    b, L = words.shape
    ntok = b * L
    kseg = (L - BURN) // NSEG            # 62
    depth = L - (NSEG - 1) * kseg        # 78
    ntokp = NSEG * kseg * b + depth * b  # padded x/H width
    w_tm = np.ascontiguousarray(words.T).reshape(-1)
    tags_tm = np.ascontiguousarray(tags.T).reshape(-1)
    m_tm = np.ascontiguousarray(mask.T).reshape(-1).astype(F32)

    d = {}
    x_full = np.asarray(emb_bf)[w_tm]                  # (ntok, 128) bf16
    xT = np.ascontiguousarray(x_full.T)                # (128, ntok)
    # macro-step-major reorder: column block (j, s) holds tokens t=kseg*s+j
    J = np.arange(depth)[:, None, None]
    S_ = np.arange(NSEG)[None, :, None]
    B_ = np.arange(b)[None, None, :]
    perm = ((kseg * S_ + J) * b + B_).reshape(-1)
    d["xdata"] = np.ascontiguousarray(xT[:, perm])     # (128, depth*NSEG*b)
    tm_masked = np.where(m_tm > 0, tags_tm, 99).astype(F32)
    d["tmask"] = tm_masked.astype(BF16).reshape(1, ntok)
    tprev = np.concatenate([np.full(b, 48, F32), tags_tm[:-b].astype(F32)])
    # pcol layouts (token%128 -> partition, token//128 -> free), host-prepped
    d["tags_pc"] = np.ascontiguousarray(tm_masked.reshape(-1, 128).T).astype(F32)
    d["tprev_pc"] = np.ascontiguousarray(tprev.reshape(-1, 128).T).astype(F32)
    d["mask_pc"] = np.ascontiguousarray(m_tm.reshape(-1, 128).T).astype(BF16)
    m_pad = np.pad(m_tm, (0, b))
    d["lsrow"] = (m_tm - m_pad[b:]).astype(BF16).reshape(1, ntok)
    return d


# --------------------------------------------------------------------------
# device kernel builder
# --------------------------------------------------------------------------

def build(L=512, BLOC=32, W=8, V=32000, debug=False, phases=("lstm", "hist", "emis", "crf")):
    ntok = L * BLOC
    half = L // 2
    NCH = ntok // 512          # emission chunks
    NPCH = ntok // 128         # one-hot pchunks
    c0 = float(np.log(T))
    KSEG = (L - BURN) // NSEG              # 62
    DEPTH = L - (NSEG - 1) * KSEG          # 78
    SPAN = NSEG * KSEG * BLOC              # 15872
    NTOKP = SPAN + DEPTH * BLOC            # 18368
    SW = NSEG * BLOC                       # 256
    GW = 4 * SW                            # 1024

    XW = DEPTH * NSEG * BLOC               # 19968, macro-major reordered x

    nc = bacc.Bacc()
    dp = nc.declare_dram_parameter
    g_x = dp("xdata", [128, XW], DT.bfloat16, isOutput=False)
    g_bias4 = {d: dp(f"bias4_{d}", [4, 128], DT.bfloat16, isOutput=False) for d in "fb"}
    g_blk1 = dp("blk1", [4, GW], DT.bfloat16, isOutput=False)
    g_tmask = dp("tmask", [1, ntok], DT.bfloat16, isOutput=False)
    g_tagspc = dp("tags_pc", [128, ntok // 128], DT.float32, isOutput=False)
    g_tprevpc = dp("tprev_pc", [128, ntok // 128], DT.float32, isOutput=False)
    g_maskpc = dp("mask_pc", [128, ntok // 128], DT.bfloat16, isOutput=False)
    g_lsrow = dp("lsrow", [1, ntok], DT.bfloat16, isOutput=False)
    g_w = {}
    for nm in ("wiht_f", "whht_f", "wiht_b", "whht_b"):
        g_w[nm] = dp(nm, [128, 512], DT.bfloat16, isOutput=False)
    g_wot = {d: dp(f"wot_{d}", [128, T], DT.bfloat16, isOutput=False) for d in "fb"}
    g_exbias = dp("exbias", [T, 1], DT.float32, isOutput=False)
    g_et = dp("et", [T, T], DT.bfloat16, isOutput=False)
    g_ett = dp("ett", [T, T], DT.bfloat16, isOutput=False)
    g_estart = dp("estart", [T, 1], DT.float32, isOutput=False)
    g_eendrow = dp("eendrow", [1, T], DT.bfloat16, isOutput=False)
    g_endrow = dp("endrow", [1, T], DT.bfloat16, isOutput=False)
    g_transext = dp("transext", [49, T], DT.float32, isOutput=False)
    g_iota48c = dp("iota48c", [T, 1], DT.float32, isOutput=False)
    g_iota64row = dp("iota64row", [1, 64], DT.float32, isOutput=False)
    g_iota48row = dp("iota48row", [1, T], DT.float32, isOutput=False)
    g_out = dp("out", [1, 8], DT.float32, isOutput=True)
    if debug:
        g_dbg1 = dp("dbg1", [T, 512], DT.float32, isOutput=True)
        g_dbg2 = dp("dbg2", [T, 512], DT.float32, isOutput=True)
        g_dbg3 = dp("dbg3", [T, 16], DT.float32, isOutput=True)

    with TileContext(nc) as tc:
        with tc.tile_pool(name="persist", bufs=1) as pp:
            # ---- persistent SBUF tiles
            Hf = pp.tile([128, NTOKP], DT.bfloat16, tag="Hf", name="Hf")
            Hb = pp.tile([128, NTOKP], DT.bfloat16, tag="Hb", name="Hb")
            wiht = {}
            whht = {}
            wot = {}
            for d in "fb":
                wiht[d] = pp.tile([128, 512], DT.bfloat16, tag=f"wiht{d}", name=f"wiht{d}")
                whht[d] = pp.tile([128, 512], DT.bfloat16, tag=f"whht{d}", name=f"whht{d}")
                wot[d] = pp.tile([128, T], DT.bfloat16, tag=f"wot{d}", name=f"wot{d}")
            exbias = pp.tile([T, 1], DT.float32, tag="exbias", name="exbias")
            et_sb = pp.tile([T, T], DT.bfloat16, tag="et", name="et")
            ett_sb = pp.tile([T, T], DT.bfloat16, tag="ett", name="ett")
            estart = pp.tile([T, 1], DT.float32, tag="estart", name="estart")
            eendrow = pp.tile([1, T], DT.bfloat16, tag="eendrow", name="eendrow")
            endrow = pp.tile([1, T], DT.bfloat16, tag="endrow", name="endrow")
            transext = pp.tile([49, T], DT.float32, tag="transext", name="transext")
            iota48c = pp.tile([T, 1], DT.float32, tag="iota48c", name="iota48c")
            iota64row = pp.tile([1, 64], DT.float32, tag="iota64row", name="iota64row")
            iota48row = pp.tile([1, T], DT.float32, tag="iota48row", name="iota48row")
            tmask_sb = pp.tile([1, ntok], DT.bfloat16, tag="tmask", name="tmask")
            tags_pcol = pp.tile([128, NPCH], DT.float32, tag="tagspcol", name="tagspcol")
            tprev_pcol = pp.tile([128, NPCH], DT.float32, tag="tprevpcol", name="tprevpcol")
            m_pcol = pp.tile([128, NPCH], DT.bfloat16, tag="mpcol", name="mpcol")
            lsrow = pp.tile([1, ntok], DT.bfloat16, tag="lsrow", name="lsrow")
            # small constants
            ones48row = pp.tile([1, T], DT.float32, tag="ones48row", name="ones48row")
            ones48rowb = pp.tile([1, T], DT.bfloat16, tag="ones48rowb", name="ones48rowb")
            ones128row = pp.tile([1, 128], DT.float32, tag="ones128row", name="ones128row")
            ones48col = pp.tile([T, 1], DT.float32, tag="ones48col", name="ones48col")
            ones49col = pp.tile([49, 1], DT.float32, tag="ones49col", name="ones49col")
            ones128col = pp.tile([128, 1], DT.float32, tag="ones128col", name="ones128col")
            iota64b = pp.tile([128, 64], DT.float32, tag="iota64b", name="iota64b")
            iota48b = pp.tile([128, T], DT.float32, tag="iota48b", name="iota48b")
            # LSTM state (fused across NSEG segments)
            cst = {d: pp.tile([128, SW], DT.float32, tag=f"c{d}", name=f"c{d}") for d in "fb"}
            tmp1 = {d: pp.tile([128, SW], DT.float32, tag=f"tmp1{d}", name=f"tmp1{d}") for d in "fb"}
            tmp2 = {d: pp.tile([128, SW], DT.bfloat16, tag=f"tmp2{d}", name=f"tmp2{d}") for d in "fb"}
            s2c = {d: pp.tile([128, SW], DT.bfloat16, tag=f"s2c{d}", name=f"s2c{d}") for d in "fb"}
            jacc = {d: pp.tile([128, 1], DT.float32, tag=f"jacc{d}", name=f"jacc{d}") for d in "fb"}
            bias4 = {d: pp.tile([4, 128], DT.bfloat16, tag=f"bias4{d}", name=f"bias4{d}") for d in "fb"}
            blk1 = pp.tile([4, GW], DT.bfloat16, tag="blk1", name="blk1")
            # numerator accumulators
            accA = pp.tile([T, NCH], DT.float32, tag="accA", name="accA")
            accA_red = pp.tile([T, 1], DT.float32, tag="accAred", name="accAred")
            accBC = pp.tile([49, 1], DT.float32, tag="accBC", name="accBC")
            junkA = pp.tile([T, 512], DT.bfloat16, tag="junkA", name="junkA")
            junkBC = pp.tile([49, T], DT.float32, tag="junkBC", name="junkBC")
            msum = pp.tile([128, 1], DT.float32, tag="msum", name="msum")
            # CRF tiles
            ea = [pp.tile([T, BLOC], DT.bfloat16, tag=f"ea{i}", name=f"ea{i}") for i in range(2)]
            emg = pp.tile([T, BLOC], DT.bfloat16, tag="emg", name="emg")
            dott = pp.tile([T, BLOC], DT.float32, tag="dott", name="dott")
            logrow = pp.tile([1, BLOC], DT.float32, tag="logrow", name="logrow")
            dsum = pp.tile([1, 1], DT.float32, tag="dsum", name="dsum")
            tmp11 = pp.tile([1, 1], DT.float32, tag="tmp11", name="tmp11")
            out_sb = pp.tile([1, 8], DT.float32, tag="outsb", name="outsb")

            # ---- input DMAs
            S = nc.sync
            for d in "fb":
                S.dma_start(out=wiht[d][:], in_=g_w[f"wiht_{d}"][:])
                S.dma_start(out=whht[d][:], in_=g_w[f"whht_{d}"][:])
                S.dma_start(out=bias4[d][:], in_=g_bias4[d][:])
                S.dma_start(out=wot[d][:], in_=g_wot[d][:])
            S.dma_start(out=blk1[:], in_=g_blk1[:])
            S.dma_start(out=exbias[:], in_=g_exbias[:])
            S.dma_start(out=et_sb[:], in_=g_et[:])
            S.dma_start(out=ett_sb[:], in_=g_ett[:])
            S.dma_start(out=estart[:], in_=g_estart[:])
            S.dma_start(out=eendrow[:], in_=g_eendrow[:])
            S.dma_start(out=endrow[:], in_=g_endrow[:])
            S.dma_start(out=transext[:], in_=g_transext[:])
            S.dma_start(out=iota48c[:], in_=g_iota48c[:])
            S.dma_start(out=iota64row[:], in_=g_iota64row[:])
            S.dma_start(out=iota48row[:], in_=g_iota48row[:])
            S.dma_start(out=tmask_sb[:], in_=g_tmask[:])
            S.dma_start(out=tags_pcol[:], in_=g_tagspc[:])
            S.dma_start(out=tprev_pcol[:], in_=g_tprevpc[:])
            S.dma_start(out=m_pcol[:], in_=g_maskpc[:])
            S.dma_start(out=lsrow[:], in_=g_lsrow[:])

            # constants
            Vv = nc.vector
            Sc = nc.scalar
            Vv.memset(ones48row[:], 1.0)
            Vv.memset(ones48rowb[:], 1.0)
            Vv.memset(ones128row[:], 1.0)
            Vv.memset(ones48col[:], 1.0)
            Vv.memset(ones49col[:], 1.0)
            Vv.memset(ones128col[:], 1.0)
            Vv.memset(accA[:], 0.0)
            Vv.memset(out_sb[:], 0.0)
            for d in "fb":
                Vv.memset(cst[d][:], 0.0)

            with tc.tile_pool(name="iotaps", bufs=1, space="PSUM") as ipsp:
                ip = ipsp.tile([128, 64], DT.float32, name="ip")
                nc.tensor.matmul(ip[:], ones128row[:], iota64row[:], start=True, stop=True)
                Vv.tensor_copy(iota64b[:], ip[:])
                ip2 = ipsp.tile([128, T], DT.float32, name="ip2")
                nc.tensor.matmul(ip2[:], ones128row[:], iota48row[:], start=True, stop=True)
                Vv.tensor_copy(iota48b[:], ip2[:])

            # ---------------- segmented LSTM ----------------
            # 2 dirs x NSEG time-segments, macro-depth DEPTH. Uniform index:
            #   f-dir: seg s at macro j handles t = KSEG*s + j
            #   b-dir: seg u at macro j handles t = L-1 - (KSEG*(7-u) + j)
            #          = KSEG*u + (DEPTH-1-j)   [stored t-major, ascending]
            # Burn-in (j<BURN) h-writes land in the next segment's output range
            # and are later overwritten by that segment's exact values.
            do_lstm = "lstm" in phases
            do_hist = "hist" in phases
            do_emis = True
            emis_lvl = 4
            do_crf = "crf" in phases
            if not do_lstm:
                Vv.memset(Hf[:], 0.0)
                Vv.memset(Hb[:], 0.0)
            Hdir = {"f": Hf, "b": Hb}

            def sview(tile, off):
                # (128, NSEG, BLOC) view: seg s at col off + s*KSEG*BLOC
                return tile[:, off:off + SPAN].rearrange(
                    "p (s n) -> p s n", s=NSEG)[:, :, 0:BLOC]

            def xoff(d, j):
                return (j if d == "f" else DEPTH - 1 - j) * BLOC

            hstage = {d: [pp.tile([128, SW], DT.bfloat16, tag=f"hs{d}{k}",
                                  name=f"hs{d}{k}") for k in range(2)]
                      for d in "fb"}
            with tc.tile_pool(name="lstm_ps", bufs=2, space="PSUM") as lpsp, \
                 tc.tile_pool(name="lstm_sb", bufs=2) as lsb_pool, \
                 tc.tile_pool(name="xpool", bufs=1) as xp:
                x = xp.tile([128, XW], DT.bfloat16, tag="x", name="x")
                S.dma_start(out=x[:], in_=g_x[:])

                def xslice(d, j):
                    # macro-group j (f) / DEPTH-1-j (b), contiguous (128, SW)
                    g = j if d == "f" else DEPTH - 1 - j
                    return x[:, g * SW:(g + 1) * SW]

                def alloc_xproj(d, j, close):
                    gp = lpsp.tile([128, GW], DT.float32, tag=f"gp{d}", name=f"gp{d}")
                    for hh in range(2):
                        nc.tensor.matmul(gp[:, hh * 512:(hh + 1) * 512],
                                         bias4[d][:], blk1[:, hh * 512:(hh + 1) * 512],
                                         start=True, stop=False, skip_group_check=True)
                    xv = xslice(d, j)
                    for c in range(4):
                        nc.tensor.matmul(
                            gp[:, c * SW:(c + 1) * SW],
                            wiht[d][:, c * 128:(c + 1) * 128], xv,
                            start=False, stop=(close and c == 3),
                            skip_group_check=True)
                    return gp

                gcur = {d: alloc_xproj(d, 0, True) for d in "fb"} if do_lstm else {}
                for j in range(DEPTH if do_lstm else 0):
                    first = (j == 0)
                    for d in "fb":
                        gp = gcur[d]
                        if not first:
                            hv = hstage[d][(j - 1) & 1]
                            for c in range(4):
                                nc.tensor.matmul(
                                    gp[:, c * SW:(c + 1) * SW],
                                    whht[d][:, c * 128:(c + 1) * 128], hv[:],
                                    start=False, stop=(c == 3),
                                    skip_group_check=True)
                    Sgt = {}
                    for d in "fb":
                        Sgd = lsb_pool.tile([128, GW], DT.bfloat16, tag=f"S{d}", name=f"S{d}")
                        Sc.activation(Sgd[:], gcur[d][:], AF.Sigmoid)
                        Sgt[d] = Sgd
                    for d in "fb":
                        Sgd = Sgt[d]
                        # gate blocks: i [0:SW], f [SW:2SW], o [2SW:3SW], g~ [3SW:4SW]
                        Vv.affine_mul_reduce(
                            out=tmp2[d][:], accum_out=jacc[d][:],
                            in0=Sgd[:, 3 * SW:4 * SW], in1=Sgd[:, 0:SW],
                            scale=2.0, bias=-1.0)
                        if first:
                            Vv.tensor_copy(cst[d][:], tmp2[d][:])
                        else:
                            Vv.tensor_tensor(out=tmp1[d][:], in0=Sgd[:, SW:2 * SW],
                                             in1=cst[d][:], op=ALU.mult)
                            Vv.tensor_tensor(out=cst[d][:], in0=tmp1[d][:],
                                             in1=tmp2[d][:], op=ALU.add)
                        Sc.activation(s2c[d][:], cst[d][:], AF.Sigmoid, scale=2.0)
                        hs = hstage[d][j & 1]
                        Vv.affine_mul_reduce(
                            out=hs[:], accum_out=jacc[d][:],
                            in0=s2c[d][:], in1=Sgd[:, 2 * SW:3 * SW],
                            scale=2.0, bias=-1.0)
                        # off-path: scatter h into t-major H for emis/CRF
                        Vv.tensor_copy(
                            sview(Hdir[d], xoff(d, j)),
                            hs[:].rearrange("p (s n) -> p s n", s=NSEG))
                        if j + 1 < DEPTH:
                            gcur[d] = alloc_xproj(d, j + 1, False)

            # ---------------- one-hot histogram (B + C numerator part) -------
            empool = tc.tile_pool(name="empool", bufs=1)
            emp = empool.__enter__()
            EM = emp.tile([T, ntok], DT.bfloat16, tag="EM", name="EM")
            if not do_emis:
                Vv.memset(EM[:], 0.01)
            with tc.tile_pool(name="cnt_ps", bufs=1, space="PSUM") as cpsp, \
                 tc.tile_pool(name="oht_sb", bufs=3) as osb:
                cntps = cpsp.tile([64, T], DT.float32, name="cntps")
                if not do_hist:
                    Vv.memset(accBC[:], 0.0)
                for q in range(NPCH if do_hist else 0):
                    ohp = osb.tile([128, 64], DT.bfloat16, tag="ohp", name="ohp")
                    ohc = osb.tile([128, T], DT.bfloat16, tag="ohc", name="ohc")
                    Vv.tensor_scalar(ohp[:], iota64b[:], tprev_pcol[:, q:q + 1],
                                     None, ALU.is_equal)
                    Vv.tensor_scalar(ohc[:], iota48b[:], tags_pcol[:, q:q + 1],
                                     None, ALU.is_equal)
                    nc.tensor.matmul(cntps[:], ohp[:], ohc[:],
                                     start=(q == 0), stop=(q == NPCH - 1),
                                     skip_group_check=True)
                if do_hist:
                    Vv.affine_mul_reduce(
                        out=junkBC[:], accum_out=accBC[:],
                        in0=transext[:], in1=cntps[0:49, :],
                        scale=1.0, bias=0.0)

            # ---------------- emissions + A-part numerator ----------------
            with tc.tile_pool(name="em_ps", bufs=2, space="PSUM") as epsp, \
                 tc.tile_pool(name="em_sb", bufs=3) as esb:
                for k in range(NCH if do_emis else 0):
                    cs = k * 512
                    emps = epsp.tile([T, 512], DT.float32, tag="emps", name="emps")
                    nc.tensor.matmul(emps[:], wot["f"][:], Hf[:, cs:cs + 512],
                                     start=True, stop=False, skip_group_check=True)
                    nc.tensor.matmul(emps[:], wot["b"][:], Hb[:, cs:cs + 512],
                                     start=False, stop=False, skip_group_check=True)
                    Sc.activation(EM[:, cs:cs + 512], emps[:], AF.Exp,
                                  bias=exbias[:])
                    if emis_lvl >= 2:
                        # fold end_trans * laststep into psum for the A-gather
                        nc.tensor.matmul(
                            emps[:], endrow[:],
                            lsrow[0:1, cs:cs + 512],
                            start=False, stop=True, skip_group_check=True)
                    if emis_lvl >= 3:
                        # tags broadcast + one-hot
                        tgps = epsp.tile([T, 512], DT.float32, tag="tgps", name="tgps")
                        nc.tensor.matmul(tgps[:], ones48rowb[:],
                                         tmask_sb[0:1, cs:cs + 512], start=True, stop=True,
                                         skip_group_check=True)
                        ohm = esb.tile([T, 512], DT.bfloat16, tag="ohm", name="ohm")
                        Vv.tensor_scalar(ohm[:], tgps[:], iota48c[:], None, ALU.is_equal)
                    if emis_lvl >= 4:
                        Vv.affine_mul_reduce(
                            out=junkA[:], accum_out=accA[:, k:k + 1],
                            in0=emps[:], in1=ohm[:],
                            scale=1.0, bias=0.0)
                    if debug and k == NCH - 1:
                        demp = esb.tile([T, 512], DT.float32, name="demp")
                        Vv.tensor_copy(demp[:], emps[:])
                        S.dma_start(out=g_dbg1[:], in_=demp[:])
                        dohm = esb.tile([T, 512], DT.float32, name="dohm")
                        Vv.tensor_copy(dohm[:], ohm[:])
                        S.dma_start(out=g_dbg2[:], in_=dohm[:])

            # ---------------- CRF ----------------
            with tc.tile_pool(name="crf_ps", bufs=2, space="PSUM") as kpsp, \
                 tc.tile_pool(name="fin_ps", bufs=1, space="PSUM") as fpsp:
                # alpha chain
                Vv.tensor_scalar(ea[0][:], EM[:, 0:BLOC], estart[:], None, ALU.mult)
                cur = 0
                for t in range(1, half if do_crf else 1):
                    pa = kpsp.tile([T, BLOC], DT.float32, tag="pa", name="pa")
                    nc.tensor.matmul(pa[:], et_sb[:], ea[cur][:], start=True, stop=True,
                                     skip_group_check=True)
                    cur ^= 1
                    Vv.tensor_tensor(out=ea[cur][:], in0=pa[:],
                                     in1=EM[:, t * BLOC:(t + 1) * BLOC], op=ALU.mult)
                # G chain: t goes L-1 down to half-1; G_t kept in psum
                def ls_slice(t):
                    tok = t * BLOC
                    return lsrow[0:1, tok:tok + BLOC]

                gps_prev = kpsp.tile([T, BLOC], DT.float32, tag="pg", name="pg")
                nc.tensor.matmul(gps_prev[:], eendrow[:], ls_slice(L - 1),
                                 start=True, stop=True, skip_group_check=True)
                for t in range(L - 2, (half - 2) if do_crf else (L - 2), -1):
                    Vv.tensor_tensor(out=emg[:], in0=gps_prev[:],
                                     in1=EM[:, (t + 1) * BLOC:(t + 2) * BLOC],
                                     op=ALU.mult)
                    gps = kpsp.tile([T, BLOC], DT.float32, tag="pg", name="pg")
                    nc.tensor.matmul(gps[:], ett_sb[:], emg[:], start=True, stop=False,
                                     skip_group_check=True)
                    nc.tensor.matmul(gps[:], eendrow[:], ls_slice(t),
                                     start=False, stop=True, skip_group_check=True)
                    gps_prev = gps
                # combine
                Vv.tensor_tensor(out=dott[:], in0=gps_prev[:], in1=ea[cur][:],
                                 op=ALU.mult)
                fint = fpsp.tile([1, 64], DT.float32, name="fint")
                nc.tensor.matmul(fint[:, 0:BLOC], ones48col[:], dott[:], start=True,
                                 stop=True, skip_group_check=True)
                Sc.activation(logrow[:], fint[:, 0:BLOC], AF.Ln)
                Vv.tensor_reduce(dsum[:], logrow[:], mybir.AxisListType.X, ALU.add)

                # masksum
                Vv.tensor_reduce(msum[:], m_pcol[:], mybir.AxisListType.X, ALU.add)
                nc.tensor.matmul(fint[:, 32:33], msum[:], ones128col[:], start=True,
                                 stop=True, skip_group_check=True)
                # numerator total: A (start) + BC (accumulate) in one cell
                Vv.tensor_reduce(accA_red[:], accA[:], mybir.AxisListType.X, ALU.add)
                nc.tensor.matmul(fint[:, 34:35], accA_red[:], ones48col[:], start=True,
                                 stop=False, skip_group_check=True)
                nc.tensor.matmul(fint[:, 34:35], accBC[:], ones49col[:], start=False,
                                 stop=True, skip_group_check=True)
                # out[0] = numsum ; out[1] = denomsum
                Vv.tensor_copy(out_sb[:, 0:1], fint[:, 34:35])
                Vv.tensor_scalar(tmp11[:], fint[:, 32:33], c0, None, ALU.mult)
                Vv.tensor_tensor(out=out_sb[:, 1:2], in0=tmp11[:], in1=dsum[:],
                                 op=ALU.add)
                Vv.tensor_copy(out_sb[:, 4:5], fint[:, 32:33])
                Vv.tensor_copy(out_sb[:, 5:6], dsum[:])
            if debug:
                daccA = pp.tile([T, 16], DT.float32, tag="daccA", name="daccA")
                Vv.memset(daccA[:], 0.0)
                Vv.tensor_copy(daccA[:, 0:NCH if NCH <= 16 else 16],
                               accA[:, 0:NCH if NCH <= 16 else 16])
                S.dma_start(out=g_dbg3[:], in_=daccA[:])
            empool.__exit__(None, None, None)
            S.dma_start(out=g_out[:], in_=out_sb[:])

    return nc


# --------------------------------------------------------------------------
# self-contained entry point: kernel(**inputs) -> scalar loss (numpy)
# --------------------------------------------------------------------------

_CACHED = {}


def _get_nc():
    if "nc" not in _CACHED:
        nc = build(L=512, BLOC=32, W=8, V=32000)
        if not nc.is_finalized():
            nc.finalize()
        _CACHED["nc"] = nc
    return _CACHED["nc"]


def kernel(**inputs):
    from concourse.bass_utils import run_bass_kernel_spmd

    B = 256
    BLOC = B // 8
    p = prep_params(inputs)
    in_maps = []
    words = np.asarray(inputs["words"])
    tags = np.asarray(inputs["tags"])
    mask = np.asarray(inputs["mask"])
    for core in range(8):
        sl = slice(core * BLOC, (core + 1) * BLOC)
        d = prep_shard(words[sl], tags[sl], mask[sl], p["emb"])
        d.update(p)
        d.pop("emb", None)
        in_maps.append(d)
    nc = _get_nc()
    res = run_bass_kernel_spmd(nc, in_maps, list(range(8)))
    tot_num = sum(float(res.results[i]["out"][0, 0]) for i in range(8))
    tot_den = sum(float(res.results[i]["out"][0, 1]) for i in range(8))
    loss = (tot_den - tot_num) / B
    return np.float32(loss)

